# revision 36
# baseline (speedup 1.0000x reference)
"""Trainium2 Bass kernel for LocalSparseAttention.

Problem (hardcoded): B=2, S=2048, D=1024, H=16, HD=64, WINDOW=128 (band
|i-j| <= 64), fp32 I/O.

Sharding: 8 cores = 2 batches x 4 head-groups (4 heads each). Each core:
  - qk projection into transposed layout [512, 2048] (head-pair packed)
  - v projection into natural layout, 17 (15 64-shifted + 2 aligned
    boundary) seq chunks, each with a ones column appended
  - banded attention: per 128-query tile, 256-key window, exp on ACT,
    0/1 band mask on DVE; AV is computed FLIPPED (masked exp weights
    stationary, v+ones moving) so each query's softmax denominator lands
    in psum column 64 and normalization is a per-partition
    reciprocal+scale on DVE; a PE identity-transpose then restores the
    [hd, seq] layout the output projection contracts over
  - output projection -> fp16 partial [2048, 1024]
Host: fp16 casts/transposes in, sum of 4 partials per batch + fused bias
(b_out + b_v @ w_out) out.

All matmuls run in fp16 (1 cycle/row on PE, ~3e-4 rel err) with fp32 PSUM
accumulation; softmax exp input stays fp32.

Input DMAs land per contraction-slice (wqk[kt]+xT[kt]) alternating across
the two HWDGE queues so arrival matches the PE's consumption rate (each
issue costs ~600ns and a queue holds ~3 in-flight DMAs); output stores
are coalesced to one [128,1024] DMA per row block, alternating between
Sync and the software-DGE gpsimd queue. Every block's exp and recip/nrm
waits are covered by independent PE filler (qk/v chunks early, deferred
out-projection tiles late); the last block runs at pair granularity with
tile-at-a-time transposes so the final stores start immediately.
"""
import sys

if "/opt/trn_rl_repo" not in sys.path:
    sys.path.insert(0, "/opt/trn_rl_repo")

import numpy as np

import concourse.bass as bass
import concourse.mybir as mybir
import concourse.tile as tile
from concourse import bacc
from concourse.bass_utils import run_bass_kernel_spmd

B, S, D, H, HD = 2, 2048, 1024, 16, 64
SCALE = HD**-0.5
C_SUB = 4.0  # subtracted from all scores via the mask; cancels in softmax

F16 = mybir.dt.float16
F32 = mybir.dt.float32

# 17 key/value chunk offsets: 15 shifted (128c+64) + aligned 0, 1920
OFFS = [128 * c + 64 for c in range(15)] + [0, 1920]


def _chunk_pair(i):
    if i == 0:
        return 15, 0
    if i == 15:
        return 14, 16
    return i - 1, i


def _build_pair_masks():
    # variant 0: (first, interior) — c4=0 pair 0
    # variant 1: (interior, interior)
    # variant 2: (interior, last)  — c4=3 pair 1
    m = _build_masks()  # [128, 3(first/int/last), 2(half), 128]
    mp = np.zeros((128, 3, 2, 2, 128), np.float16)
    mp[:, 0, 0] = m[:, 0]
    mp[:, 0, 1] = m[:, 1]
    mp[:, 1, 0] = m[:, 1]
    mp[:, 1, 1] = m[:, 1]
    mp[:, 2, 0] = m[:, 1]
    mp[:, 2, 1] = m[:, 2]
    return mp


def _build_masks():
    kp = np.arange(128)[:, None]  # key within chunk (partition)
    p = np.arange(128)[None, :]  # query within tile
    masks = np.zeros((128, 3, 2, 128), np.float16)
    # interior tile i: half0 = chunk@i*128-64, half1 = chunk@i*128+64
    masks[:, 1, 0] = (np.abs(p + 64 - kp) <= 64).astype(np.float16)
    masks[:, 1, 1] = (np.abs(p - 64 - kp) <= 64).astype(np.float16)
    # first tile (i=0): half0 = aligned@0 (owns keys < 64), half1 = chunk@64
    masks[:, 0, 0] = ((kp < 64) & (np.abs(p - kp) <= 64)).astype(np.float16)
    masks[:, 0, 1] = masks[:, 1, 1]
    # last tile (i=15): half0 = chunk@1856 (owns keys < 1920), half1 = @1920
    masks[:, 2, 0] = ((kp < 64) & (np.abs(p + 64 - kp) <= 64)).astype(
        np.float16)
    masks[:, 2, 1] = (np.abs(p - kp) <= 64).astype(np.float16)
    return masks


def _build_program():
    nc = bacc.Bacc("TRN2", debug=False, num_devices=8)

    xT_d = nc.dram_tensor("xT", [D, S], F16, kind="ExternalInput").ap()
    wqk_d = nc.dram_tensor("wqk", [D, 512], F16, kind="ExternalInput").ap()
    wv_d = nc.dram_tensor("wv", [D, 256], F16, kind="ExternalInput").ap()
    wout_d = nc.dram_tensor("wout", [256, D], F16, kind="ExternalInput").ap()
    bqk_d = nc.dram_tensor("bqk", [128, 4], F32, kind="ExternalInput").ap()
    masks_d = nc.dram_tensor("masks", [128, 3, 2, 2, 128], F16,
                             kind="ExternalInput").ap()
    ident_d = nc.dram_tensor("ident", [128, 128], F16,
                             kind="ExternalInput").ap()
    out_d = nc.dram_tensor("out", [S, D], F16, kind="ExternalOutput").ap()

    with tile.TileContext(nc) as tc:
        with (
            tc.tile_pool(name="const", bufs=1) as cpool,
            tc.tile_pool(name="work", bufs=2) as wpool,
            tc.tile_pool(name="expp", bufs=10) as epool,
            tc.tile_pool(name="ysb", bufs=4) as ypool,
            tc.tile_pool(name="ps512", bufs=2, space="PSUM") as ps512,
            tc.tile_pool(name="psv", bufs=2, space="PSUM") as psv,
            tc.tile_pool(name="pssc", bufs=2, space="PSUM") as pssc,
            tc.tile_pool(name="psav", bufs=2, space="PSUM") as psav,
        ):
            # ---- persistent SBUF tensors ----
            xT_sb = cpool.tile([128, 8, S], F16, tag="xT")
            wqk_sb = cpool.tile([128, 8, 512], F16, tag="wqk")
            wv_sb = cpool.tile([128, 8, 256], F16, tag="wv")
            wout_sb = cpool.tile([128, 2, D], F16, tag="wout")
            bqk_sb = cpool.tile([128, 4], F32, tag="bqk")
            masks_sb = cpool.tile([128, 3, 2, 2, 128], F16, tag="masks")
            qk_sb = cpool.tile([128, 4, S], F16, tag="qk")
            v_sb = cpool.tile([128, 17, 4, 65], F16, tag="v")
            aoT_sb = cpool.tile([128, 2, S], F16, tag="aoT")
            negc_sb = cpool.tile([128, 1], F32, tag="negc")
            ident_sb = cpool.tile([128, 128], F16, tag="ident")

            # ---- input DMAs: few large issues spread over three queues
            # (each HWDGE queue holds ~3 in-flight DMAs and an issue costs
            # ~600ns-1us; a 4th issue blocks the engine, so keep <=3 per
            # queue and put slack tensors on the software-DGE gpsimd
            # queue) ----
            xT_r = xT_d.rearrange("(ko kp) s -> kp ko s", kp=128)
            wsrc = cpool.tile([128, 512], F16, tag="wsrc")
            wdst = cpool.tile([128, 512], F16, tag="wdst")
            nc.vector.memset(wsrc[:], 0.0)
            # per-kt chunks interleaved across the two HWDGE queues so the
            # kt-th contraction slice (wqk[kt] + xT[kt]) lands every
            # ~1.8us, matching the PE's kt consumption rate; excess issues
            # beyond queue depth 3 just pace the stream. wv/masks/wout ride
            # at the back of the queues (needed only ~25us+ in).
            wqk_r = wqk_d.rearrange("(ko kp) n -> kp ko n", kp=128)
            for kt in range(8):
                eng = nc.scalar if kt % 2 == 0 else nc.sync
                eng.dma_start(out=wqk_sb[:, kt], in_=wqk_r[:, kt])
                eng.dma_start(out=xT_sb[:, kt], in_=xT_r[:, kt])
            nc.sync.dma_start(
                out=wv_sb[:],
                in_=wv_d.rearrange("(ko kp) n -> kp ko n", kp=128),
            )
            nc.scalar.dma_start(out=masks_sb[:], in_=masks_d)
            nc.sync.dma_start(
                out=wout_sb[:],
                in_=wout_d.rearrange("(t p) n -> p t n", p=128),
            )
            nc.gpsimd.dma_start(out=bqk_sb[:], in_=bqk_d)
            nc.gpsimd.dma_start(out=ident_sb[:], in_=ident_d)
            nc.vector.memset(negc_sb[:], -C_SUB)
            nc.vector.memset(v_sb[:, :, :, 64:65], 1.0)

            # ---- PE warmup: dummy matmuls on zeroed SBUF so the HAM
            # clock-gate ramps before the first real matmul (which waits
            # for input DMA) ----
            wps = ps512.tile([128, 512], F32, tag="ps512")
            for w in range(8):
                nc.tensor.matmul(
                    out=wps[:],
                    lhsT=wsrc[:, 0:128],
                    rhs=wsrc[:],
                    start=(w == 0),
                    stop=(w == 7),
                )
            nc.scalar.copy(out=wdst[:], in_=wps[:])

            # ---- emission helpers (B work interleaved into C keeps the
            # PE array duty high so HAM stays at full clock) ----
            def emit_qk_chunk(ns, pools=None):
                # all 4 m-tiles of q/k projection for seq chunk ns.
                # `pools` round-robins PSUM tags so many accumulation
                # groups stay open while xT chunks stream in.
                for m in range(4):
                    scale = SCALE if m < 2 else 1.0
                    if pools is None:
                        ps = ps512.tile([128, 512], F32, tag="ps512")
                    else:
                        pool, tg = pools[m % len(pools)]
                        ps = pool.tile([128, 512], F32, tag=tg)
                    for kt in range(8):
                        nc.tensor.matmul(
                            out=ps[:],
                            lhsT=wqk_sb[:, kt, m * 128:(m + 1) * 128],
                            rhs=xT_sb[:, kt, ns * 512:(ns + 1) * 512],
                            start=(kt == 0),
                            stop=(kt == 7),
                        )
                    nc.scalar.activation(
                        out=qk_sb[:, m, ns * 512:(ns + 1) * 512],
                        in_=ps[:],
                        func=mybir.ActivationFunctionType.Identity,
                        bias=bqk_sb[:, m:m + 1],
                        scale=scale,
                    )

            def emit_v_chunk(c, pool_tag=None):
                off = OFFS[c]
                if pool_tag is None:
                    ps = psv.tile([128, 256], F32, tag="psv")
                else:
                    pool, tg = pool_tag
                    ps = pool.tile([128, 256], F32, tag=tg)
                for kt in range(8):
                    nc.tensor.matmul(
                        out=ps[:],
                        lhsT=xT_sb[:, kt, off:off + 128],
                        rhs=wv_sb[:, kt, :],
                        start=(kt == 0),
                        stop=(kt == 7),
                    )
                if c % 2 == 0:
                    nc.scalar.copy(
                        out=v_sb[:, c, :, 0:64],
                        in_=ps[:].rearrange("p (h d) -> p h d", h=4),
                    )
                else:
                    nc.vector.tensor_copy(
                        out=v_sb[:, c, :, 0:64],
                        in_=ps[:].rearrange("p (h d) -> p h d", h=4),
                    )

            def emit_scores(c4, hp, pairs=(0, 1)):
                # scores + exp for both heads of the pair
                ex_big = {}
                for hh in range(2):
                    ex_big[hh] = epool.tile(
                        [128, 2 * len(pairs), 2, 128], F16, tag="exp",
                        name="exb")
                for pi, pair in enumerate(pairs):
                    if c4 == 0 and pair == 0:
                        pv = 0
                    elif c4 == 3 and pair == 1:
                        pv = 2
                    else:
                        pv = 1
                    sc_h0 = pssc.tile([128, 2, 2, 128], F32, tag="pssc")
                    sc_h1 = pssc.tile([128, 2, 2, 128], F32, tag="pssc")
                    scs = {0: sc_h0, 1: sc_h1}
                    for iw in range(2):
                        i = c4 * 4 + pair * 2 + iw
                        cA, cB = _chunk_pair(i)
                        for hh in range(2):
                            po = hh * 64
                            for half, cc in enumerate((cA, cB)):
                                off = OFFS[cc]
                                nc.tensor.matmul(
                                    out=scs[hh][:, iw, half, :],
                                    lhsT=qk_sb[po:po + 64, 2 + hp,
                                               off:off + 128],
                                    rhs=qk_sb[po:po + 64, hp,
                                              i * 128:(i + 1) * 128],
                                    start=(iw == 0 and half == 0),
                                    stop=(iw == 1 and half == 1),
                                )
                    for hh in range(2):
                        # exp(score - C) on ACT (one op per ii-pair),
                        # band-zeroing via 0/1 mask multiply on DVE
                        sl = slice(pi * 2, pi * 2 + 2)
                        nc.scalar.activation(
                            out=ex_big[hh][:, sl],
                            in_=scs[hh][:],
                            func=mybir.ActivationFunctionType.Exp,
                            bias=negc_sb[:],
                        )
                        nc.vector.tensor_mul(
                            out=ex_big[hh][:, sl],
                            in0=ex_big[hh][:, sl],
                            in1=masks_sb[:, pv],
                        )
                return ex_big

            def emit_av(c4, hp, ex_big, pairs=(0, 1)):
                # flipped AV: the masked exp weights are the stationary
                # operand, v (with its ones column) moves -> out [q, hd+1]
                # so each query's softmax denominator lands in column 64
                # and normalization is a per-partition scalar multiply.
                nq = 2 * len(pairs)
                avq = {}
                for hh in range(2):
                    h = 2 * hp + hh
                    a = psav.tile([128, nq, 65], F32, tag="psav",
                                  name="avq")
                    avq[hh] = a
                    for qi in range(nq):
                        cA, cB = _chunk_pair(c4 * 4 + pairs[0] * 2 + qi)
                        for half, cc in enumerate((cA, cB)):
                            nc.tensor.matmul(
                                out=a[:, qi, :],
                                lhsT=ex_big[hh][:, qi, half, :],
                                rhs=v_sb[:, cc, h, 0:65],
                                start=(qi == 0 and half == 0),
                                stop=(qi == nq - 1 and half == 1),
                            )
                nrm = wpool.tile([128, 4, 128], F16, tag="nrm")
                rec = wpool.tile([128, 4, 2, 1], F32, tag="rec")
                for hh in range(2):
                    nc.vector.reciprocal_approx_fast(
                        out=rec[:, 0:nq, hh, :],
                        in_=avq[hh][:, :, 64:65],
                    )
                for qi in range(nq):
                    for hh in range(2):
                        nc.vector.tensor_scalar_mul(
                            nrm[:, qi, hh * 64:(hh + 1) * 64],
                            avq[hh][:, qi, 0:64],
                            rec[:, qi, hh, :],
                        )
                return nrm

            def emit_transp(c4, hp, nrm, pairs=(0, 1), qis=None):
                # [q, hd] -> [hd, q] via PE identity-transpose, landing in
                # the aoT layout the output projection contracts over
                if qis is None:
                    qis = range(2 * len(pairs))
                for qi in qis:
                    i = c4 * 4 + pairs[0] * 2 + qi
                    pt = psv.tile([128, 128], F16, tag="psv", name="ptr")
                    nc.tensor.matmul(
                        out=pt[:], lhsT=nrm[:, qi, :], rhs=ident_sb[:],
                        is_transpose=True, start=True, stop=True,
                    )
                    if qi % 2 == 0:
                        nc.scalar.copy(
                            out=aoT_sb[:, hp, i * 128:(i + 1) * 128],
                            in_=pt[:])
                    else:
                        nc.vector.tensor_copy(
                            out=aoT_sb[:, hp, i * 128:(i + 1) * 128],
                            in_=pt[:])

            def emit_outproj_st(st, fast_dma=False):
                # one coalesced [128,1024] store per row block (halves the
                # DMA count and semaphore-recycle churn); stores alternate
                # between the Sync HWDGE queue and the idle software-DGE
                # (gpsimd) queue. Completion only gates ysb reuse 4 tiles
                # later. Final stores take the fast Sync path.
                deng = nc.sync if (fast_dma or st % 2 == 0) else nc.gpsimd
                ysb = ypool.tile([128, 2, 512], F16, tag="ysb")
                for nn in range(2):
                    ps = ps512.tile([128, 512], F32, tag="ps512")
                    for hp2 in range(2):
                        nc.tensor.matmul(
                            out=ps[:],
                            lhsT=aoT_sb[:, hp2, st * 128:(st + 1) * 128],
                            rhs=wout_sb[:, hp2,
                                        nn * 512:(nn + 1) * 512],
                            start=(hp2 == 0),
                            stop=(hp2 == 1),
                        )
                    if nn == 0:
                        nc.scalar.copy(out=ysb[:, nn], in_=ps[:])
                    else:
                        nc.vector.tensor_copy(out=ysb[:, nn], in_=ps[:])
                deng.dma_start(
                    out=out_d[st * 128:(st + 1) * 128, :],
                    in_=ysb[:].rearrange("p a b -> p (a b)"),
                )

            # ---- emission schedule: prologue B; per pair-block the PE
            # stream is [scores | filler (prev-block outproj) | AV | B-next
            # chunks] so the PE has independent work while ACT runs exp ----
            # qk chunks 0+1 are emitted kt-major with 8 simultaneously open
            # PSUM groups so the in-order PE consumes each xT k-chunk as it
            # lands instead of stalling a whole m-tile on the last chunk.
            slots = [(ps512, "ps512"), (ps512, "ps512"),
                     (psv, "psv"), (psv, "psv"),
                     (pssc, "pssc"), (pssc, "pssc"),
                     (psav, "psav"), (psav, "psav")]
            qkps = {}
            for ns in range(2):
                for m in range(4):
                    pool, tg = slots[ns * 4 + m]
                    qkps[(ns, m)] = pool.tile([128, 512], F32, tag=tg,
                                              name="qkps")
            for kt in range(8):
                for ns in range(2):
                    for m in range(4):
                        nc.tensor.matmul(
                            out=qkps[(ns, m)][:],
                            lhsT=wqk_sb[:, kt, m * 128:(m + 1) * 128],
                            rhs=xT_sb[:, kt, ns * 512:(ns + 1) * 512],
                            start=(kt == 0),
                            stop=(kt == 7),
                        )
            for ns in range(2):
                for m in range(4):
                    nc.scalar.activation(
                        out=qk_sb[:, m, ns * 512:(ns + 1) * 512],
                        in_=qkps[(ns, m)][:],
                        func=mybir.ActivationFunctionType.Identity,
                        bias=bqk_sb[:, m:m + 1],
                        scale=(SCALE if m < 2 else 1.0),
                    )
            vrr = [(psv, "psv"), (psav, "psav"), (psv, "psv"),
                   (pssc, "pssc"), (psv, "psv")]
            for c, pt in zip((15, 0, 1, 2, 3), vrr):
                emit_v_chunk(c, pool_tag=pt)

            # mid-fillers sized so every block's exp/AV wait is covered by
            # independent PE work; out-projection tiles are deferred toward
            # the late blocks (which have no qk/v B-work left)
            midqk = {(0, 0): [2], (1, 0): [3]}
            midv = {(0, 1): [4, 5, 6, 7], (1, 1): [8, 9, 10, 11],
                    (2, 0): [12, 13], (3, 0): [14, 16]}
            # mid1 fills the scores->exp wait; mid2 fills the recip/nrm
            # wait between AV and the transposes
            mid1 = {(2, 0): [2], (2, 1): [4, 5], (3, 0): [7]}
            mid2 = {(1, 0): [0], (1, 1): [1], (2, 0): [3],
                    (2, 1): [6], (3, 0): [8]}
            for c4 in range(4):
                for hp in range(2):
                    if (c4, hp) == (3, 1):
                        continue
                    ex_big = emit_scores(c4, hp)
                    for qc in midqk.get((c4, hp), []):
                        emit_qk_chunk(qc)
                    for vc in midv.get((c4, hp), []):
                        emit_v_chunk(vc)
                    for st in mid1.get((c4, hp), []):
                        emit_outproj_st(st)
                    nrm = emit_av(c4, hp, ex_big)
                    for st in mid2.get((c4, hp), []):
                        emit_outproj_st(st)
                    emit_transp(c4, hp, nrm)
            # last block at pair granularity; the final transposes go
            # tile-at-a-time so each out-projection row block starts the
            # moment its aoT strip exists
            ex_a = emit_scores(3, 1, pairs=(0,))
            emit_outproj_st(9)
            nrm_a = emit_av(3, 1, ex_a, pairs=(0,))
            emit_outproj_st(10)
            emit_transp(3, 1, nrm_a, pairs=(0,))
            ex_b = emit_scores(3, 1, pairs=(1,))
            emit_outproj_st(11)
            emit_outproj_st(12)
            nrm_b = emit_av(3, 1, ex_b, pairs=(1,))
            emit_outproj_st(13, fast_dma=True)
            emit_transp(3, 1, nrm_b, pairs=(1,), qis=[0])
            emit_outproj_st(14, fast_dma=True)
            emit_transp(3, 1, nrm_b, pairs=(1,), qis=[1])
            emit_outproj_st(15, fast_dma=True)

    nc.compile()
    return nc


_NC = None


def _get_program():
    global _NC
    if _NC is None:
        _NC = _build_program()
    return _NC


def _make_in_maps(x, w_qkv, b_qkv, w_out):
    masks = _build_pair_masks()

    in_maps = []
    for c in range(8):
        b, hg = divmod(c, 4)
        cq = 256 * hg
        wqk = np.concatenate(
            [w_qkv[:, cq:cq + 256], w_qkv[:, 1024 + cq:1024 + cq + 256]],
            axis=1,
        ).astype(np.float16)
        bqk = np.empty((128, 4), np.float32)
        bqk[:, 0] = b_qkv[cq:cq + 128] * SCALE
        bqk[:, 1] = b_qkv[cq + 128:cq + 256] * SCALE
        bqk[:, 2] = b_qkv[1024 + cq:1024 + cq + 128]
        bqk[:, 3] = b_qkv[1024 + cq + 128:1024 + cq + 256]
        in_maps.append({
            "xT": np.ascontiguousarray(x[b].T).astype(np.float16),
            "wqk": wqk,
            "wv": w_qkv[:, 2048 + cq:2048 + cq + 256].astype(np.float16),
            "wout": w_out[cq:cq + 256, :].astype(np.float16),
            "bqk": bqk,
            "masks": masks,
            "ident": np.eye(128, dtype=np.float16),
        })
    return in_maps


def kernel(x, w_qkv, b_qkv, w_out, b_out):
    x = np.asarray(x, np.float32)
    w_qkv = np.asarray(w_qkv, np.float32)
    b_qkv = np.asarray(b_qkv, np.float32)
    w_out = np.asarray(w_out, np.float32)
    b_out = np.asarray(b_out, np.float32)

    in_maps = _make_in_maps(x, w_qkv, b_qkv, w_out)
    nc = _get_program()
    res = run_bass_kernel_spmd(nc, in_maps, list(range(8)))

    b_v = b_qkv[2048:]
    bias_all = b_out + b_v @ w_out  # folds the (untracked) v-bias
    y = np.empty((B, S, D), np.float32)
    for b in range(B):
        acc = np.zeros((S, D), np.float32)
        for hg in range(4):
            acc += res.results[4 * b + hg]["out"].astype(np.float32)
        y[b] = acc + bias_all
    return y


# revision 39
# speedup vs baseline: 1.1907x; 1.1907x over previous
"""Trainium2 Bass kernel for LocalSparseAttention.

Problem (hardcoded): B=2, S=2048, D=1024, H=16, HD=64, WINDOW=128 (band
|i-j| <= 64), fp32 I/O.

Sharding: 8 cores = 2 batches x 4 head-groups (4 heads each). Each core:
  - qk projection into transposed layout [512, 2048] (head-pair packed)
  - v projection into natural layout, 17 (15 64-shifted + 2 aligned
    boundary) seq chunks, each with a ones column appended
  - banded attention: per 128-query tile, 256-key window, exp on ACT,
    0/1 band mask on DVE; AV is computed FLIPPED (masked exp weights
    stationary, v+ones moving) so each query's softmax denominator lands
    in psum column 64 and normalization is a per-partition
    reciprocal+scale on DVE; a PE identity-transpose then restores the
    [hd, seq] layout the output projection contracts over
  - output projection -> fp16 partial [2048, 1024]
Host: fp16 casts/transposes in, sum of 4 partials per batch + fused bias
(b_out + b_v @ w_out) out.

All matmuls run in fp16 (1 cycle/row on PE, ~3e-4 rel err) with fp32 PSUM
accumulation; softmax exp input stays fp32.

Input DMAs land per contraction-slice (wqk[kt]+xT[kt]) alternating across
the two HWDGE queues so arrival matches the PE's consumption rate (each
issue costs ~600ns and a queue holds ~3 in-flight DMAs); output stores
are coalesced to one [128,1024] DMA per row block, alternating between
Sync and the software-DGE gpsimd queue. Every block's exp and recip/nrm
waits are covered by independent PE filler (qk/v chunks early, deferred
out-projection tiles late); the last block runs at pair granularity with
tile-at-a-time transposes so the final stores start immediately.
"""
import sys

if "/opt/trn_rl_repo" not in sys.path:
    sys.path.insert(0, "/opt/trn_rl_repo")

import numpy as np

import concourse.bass as bass
import concourse.mybir as mybir
import concourse.tile as tile
from concourse import bacc
from concourse.bass_utils import run_bass_kernel_spmd

B, S, D, H, HD = 2, 2048, 1024, 16, 64
SCALE = HD**-0.5
C_SUB = 4.0  # subtracted from all scores via the mask; cancels in softmax

F16 = mybir.dt.float16
F32 = mybir.dt.float32

# 17 key/value chunk offsets: 15 shifted (128c+64) + aligned 0, 1920
OFFS = [128 * c + 64 for c in range(15)] + [0, 1920]


def _chunk_pair(i):
    if i == 0:
        return 15, 0
    if i == 15:
        return 14, 16
    return i - 1, i


def _build_pair_masks():
    # variant 0: (first, interior) — c4=0 pair 0
    # variant 1: (interior, interior)
    # variant 2: (interior, last)  — c4=3 pair 1
    m = _build_masks()  # [128, 3(first/int/last), 2(half), 128]
    mp = np.zeros((128, 3, 2, 2, 128), np.float16)
    mp[:, 0, 0] = m[:, 0]
    mp[:, 0, 1] = m[:, 1]
    mp[:, 1, 0] = m[:, 1]
    mp[:, 1, 1] = m[:, 1]
    mp[:, 2, 0] = m[:, 1]
    mp[:, 2, 1] = m[:, 2]
    return mp


def _build_masks():
    kp = np.arange(128)[:, None]  # key within chunk (partition)
    p = np.arange(128)[None, :]  # query within tile
    masks = np.zeros((128, 3, 2, 128), np.float16)
    # interior tile i: half0 = chunk@i*128-64, half1 = chunk@i*128+64
    masks[:, 1, 0] = (np.abs(p + 64 - kp) <= 64).astype(np.float16)
    masks[:, 1, 1] = (np.abs(p - 64 - kp) <= 64).astype(np.float16)
    # first tile (i=0): half0 = aligned@0 (owns keys < 64), half1 = chunk@64
    masks[:, 0, 0] = ((kp < 64) & (np.abs(p - kp) <= 64)).astype(np.float16)
    masks[:, 0, 1] = masks[:, 1, 1]
    # last tile (i=15): half0 = chunk@1856 (owns keys < 1920), half1 = @1920
    masks[:, 2, 0] = ((kp < 64) & (np.abs(p + 64 - kp) <= 64)).astype(
        np.float16)
    masks[:, 2, 1] = (np.abs(p - kp) <= 64).astype(np.float16)
    return masks


def _build_program():
    nc = bacc.Bacc("TRN2", debug=False, num_devices=8)

    xT_d = nc.dram_tensor("xT", [D, S], F16, kind="ExternalInput").ap()
    wqk_d = nc.dram_tensor("wqk", [D, 512], F16, kind="ExternalInput").ap()
    wv_d = nc.dram_tensor("wv", [D, 256], F16, kind="ExternalInput").ap()
    wout_d = nc.dram_tensor("wout", [256, D], F16, kind="ExternalInput").ap()
    bqk_d = nc.dram_tensor("bqk", [128, 4], F32, kind="ExternalInput").ap()
    masks_d = nc.dram_tensor("masks", [128, 3, 2, 2, 128], F16,
                             kind="ExternalInput").ap()
    ident_d = nc.dram_tensor("ident", [128, 128], F16,
                             kind="ExternalInput").ap()
    out_d = nc.dram_tensor("out", [S, D], F16, kind="ExternalOutput").ap()

    with tile.TileContext(nc) as tc:
        with (
            tc.tile_pool(name="const", bufs=1) as cpool,
            tc.tile_pool(name="work", bufs=2) as wpool,
            tc.tile_pool(name="expp", bufs=10) as epool,
            tc.tile_pool(name="ysb", bufs=4) as ypool,
            tc.tile_pool(name="ps512", bufs=2, space="PSUM") as ps512,
            tc.tile_pool(name="psv", bufs=2, space="PSUM") as psv,
            tc.tile_pool(name="pssc", bufs=2, space="PSUM") as pssc,
            tc.tile_pool(name="psav", bufs=2, space="PSUM") as psav,
        ):
            # ---- persistent SBUF tensors ----
            xT_sb = cpool.tile([128, 8, S], F16, tag="xT")
            wqk_sb = cpool.tile([128, 8, 512], F16, tag="wqk")
            wv_sb = cpool.tile([128, 8, 256], F16, tag="wv")
            wout_sb = cpool.tile([128, 2, D], F16, tag="wout")
            bqk_sb = cpool.tile([128, 4], F32, tag="bqk")
            masks_sb = cpool.tile([128, 3, 2, 2, 128], F16, tag="masks")
            qk_sb = cpool.tile([128, 4, S], F16, tag="qk")
            v_sb = cpool.tile([128, 17, 4, 65], F16, tag="v")
            aoT_sb = cpool.tile([128, 2, S], F16, tag="aoT")
            negc_sb = cpool.tile([128, 1], F32, tag="negc")
            ident_sb = cpool.tile([128, 128], F16, tag="ident")

            # ---- input DMAs: few large issues spread over three queues
            # (each HWDGE queue holds ~3 in-flight DMAs and an issue costs
            # ~600ns-1us; a 4th issue blocks the engine, so keep <=3 per
            # queue and put slack tensors on the software-DGE gpsimd
            # queue) ----
            xT_r = xT_d.rearrange("(ko kp) s -> kp ko s", kp=128)
            wsrc = cpool.tile([128, 512], F16, tag="wsrc")
            wdst = cpool.tile([128, 512], F16, tag="wdst")
            nc.vector.memset(wsrc[:], 0.0)
            # per-kt chunks interleaved across the two HWDGE queues so the
            # kt-th contraction slice (wqk[kt] + xT[kt]) lands every
            # ~1.8us, matching the PE's kt consumption rate; excess issues
            # beyond queue depth 3 just pace the stream. wv/masks/wout ride
            # at the back of the queues (needed only ~25us+ in).
            wqk_r = wqk_d.rearrange("(ko kp) n -> kp ko n", kp=128)
            for kt in range(8):
                eng = nc.scalar if kt % 2 == 0 else nc.sync
                eng.dma_start(out=wqk_sb[:, kt], in_=wqk_r[:, kt])
                eng.dma_start(out=xT_sb[:, kt], in_=xT_r[:, kt])
            nc.sync.dma_start(
                out=wv_sb[:],
                in_=wv_d.rearrange("(ko kp) n -> kp ko n", kp=128),
            )
            nc.scalar.dma_start(out=masks_sb[:], in_=masks_d)
            nc.sync.dma_start(
                out=wout_sb[:],
                in_=wout_d.rearrange("(t p) n -> p t n", p=128),
            )
            nc.gpsimd.dma_start(out=bqk_sb[:], in_=bqk_d)
            nc.gpsimd.dma_start(out=ident_sb[:], in_=ident_d)
            nc.vector.memset(negc_sb[:], -C_SUB)
            nc.vector.memset(v_sb[:, :, :, 64:65], 1.0)

            # ---- PE warmup: dummy matmuls on zeroed SBUF so the HAM
            # clock-gate ramps before the first real matmul (which waits
            # for input DMA) ----
            wps = ps512.tile([128, 512], F32, tag="ps512")
            for w in range(8):
                nc.tensor.matmul(
                    out=wps[:],
                    lhsT=wsrc[:, 0:128],
                    rhs=wsrc[:],
                    start=(w == 0),
                    stop=(w == 7),
                )
            nc.scalar.copy(out=wdst[:], in_=wps[:])

            # ---- emission helpers (B work interleaved into C keeps the
            # PE array duty high so HAM stays at full clock) ----
            def emit_qk_chunk(ns, pools=None):
                # all 4 m-tiles of q/k projection for seq chunk ns.
                # `pools` round-robins PSUM tags so many accumulation
                # groups stay open while xT chunks stream in.
                for m in range(4):
                    scale = SCALE if m < 2 else 1.0
                    if pools is None:
                        ps = ps512.tile([128, 512], F32, tag="ps512")
                    else:
                        pool, tg = pools[m % len(pools)]
                        ps = pool.tile([128, 512], F32, tag=tg)
                    for kt in range(8):
                        nc.tensor.matmul(
                            out=ps[:],
                            lhsT=wqk_sb[:, kt, m * 128:(m + 1) * 128],
                            rhs=xT_sb[:, kt, ns * 512:(ns + 1) * 512],
                            start=(kt == 0),
                            stop=(kt == 7),
                        )
                    nc.scalar.activation(
                        out=qk_sb[:, m, ns * 512:(ns + 1) * 512],
                        in_=ps[:],
                        func=mybir.ActivationFunctionType.Identity,
                        bias=bqk_sb[:, m:m + 1],
                        scale=scale,
                    )

            def emit_v_chunk(c, pool_tag=None):
                off = OFFS[c]
                if pool_tag is None:
                    ps = psv.tile([128, 256], F32, tag="psv")
                else:
                    pool, tg = pool_tag
                    ps = pool.tile([128, 256], F32, tag=tg)
                for kt in range(8):
                    nc.tensor.matmul(
                        out=ps[:],
                        lhsT=xT_sb[:, kt, off:off + 128],
                        rhs=wv_sb[:, kt, :],
                        start=(kt == 0),
                        stop=(kt == 7),
                    )
                if c % 2 == 0:
                    nc.scalar.copy(
                        out=v_sb[:, c, :, 0:64],
                        in_=ps[:].rearrange("p (h d) -> p h d", h=4),
                    )
                else:
                    nc.vector.tensor_copy(
                        out=v_sb[:, c, :, 0:64],
                        in_=ps[:].rearrange("p (h d) -> p h d", h=4),
                    )

            def emit_scores(c4, hp, pairs=(0, 1)):
                # scores + exp for both heads of the pair
                ex_big = {}
                for hh in range(2):
                    ex_big[hh] = epool.tile(
                        [128, 2 * len(pairs), 2, 128], F16, tag="exp",
                        name="exb")
                for pi, pair in enumerate(pairs):
                    if c4 == 0 and pair == 0:
                        pv = 0
                    elif c4 == 3 and pair == 1:
                        pv = 2
                    else:
                        pv = 1
                    sc_h0 = pssc.tile([128, 2, 2, 128], F32, tag="pssc")
                    sc_h1 = pssc.tile([128, 2, 2, 128], F32, tag="pssc")
                    scs = {0: sc_h0, 1: sc_h1}
                    for iw in range(2):
                        i = c4 * 4 + pair * 2 + iw
                        cA, cB = _chunk_pair(i)
                        for hh in range(2):
                            po = hh * 64
                            for half, cc in enumerate((cA, cB)):
                                off = OFFS[cc]
                                nc.tensor.matmul(
                                    out=scs[hh][:, iw, half, :],
                                    lhsT=qk_sb[po:po + 64, 2 + hp,
                                               off:off + 128],
                                    rhs=qk_sb[po:po + 64, hp,
                                              i * 128:(i + 1) * 128],
                                    start=(iw == 0 and half == 0),
                                    stop=(iw == 1 and half == 1),
                                )
                    for hh in range(2):
                        # exp(score - C) on ACT (one op per ii-pair),
                        # band-zeroing via 0/1 mask multiply on DVE
                        sl = slice(pi * 2, pi * 2 + 2)
                        nc.scalar.activation(
                            out=ex_big[hh][:, sl],
                            in_=scs[hh][:],
                            func=mybir.ActivationFunctionType.Exp,
                            bias=negc_sb[:],
                        )
                        nc.vector.tensor_mul(
                            out=ex_big[hh][:, sl],
                            in0=ex_big[hh][:, sl],
                            in1=masks_sb[:, pv],
                        )
                return ex_big

            def emit_av(c4, hp, ex_big, pairs=(0, 1)):
                # flipped AV: the masked exp weights are the stationary
                # operand, v (with its ones column) moves -> out [q, hd+1]
                # so each query's softmax denominator lands in column 64
                # and normalization is a per-partition scalar multiply.
                nq = 2 * len(pairs)
                avq = {}
                for hh in range(2):
                    h = 2 * hp + hh
                    a = psav.tile([128, nq, 65], F32, tag="psav",
                                  name="avq")
                    avq[hh] = a
                    for qi in range(nq):
                        cA, cB = _chunk_pair(c4 * 4 + pairs[0] * 2 + qi)
                        for half, cc in enumerate((cA, cB)):
                            nc.tensor.matmul(
                                out=a[:, qi, :],
                                lhsT=ex_big[hh][:, qi, half, :],
                                rhs=v_sb[:, cc, h, 0:65],
                                start=(qi == 0 and half == 0),
                                stop=(qi == nq - 1 and half == 1),
                            )
                nrm = wpool.tile([128, 4, 128], F16, tag="nrm")
                rec = wpool.tile([128, 4, 2, 1], F32, tag="rec")
                for hh in range(2):
                    nc.vector.reciprocal_approx_fast(
                        out=rec[:, 0:nq, hh, :],
                        in_=avq[hh][:, :, 64:65],
                    )
                for qi in range(nq):
                    for hh in range(2):
                        nc.vector.tensor_scalar_mul(
                            nrm[:, qi, hh * 64:(hh + 1) * 64],
                            avq[hh][:, qi, 0:64],
                            rec[:, qi, hh, :],
                        )
                return nrm

            def emit_transp(c4, hp, nrm, pairs=(0, 1), qis=None):
                # [q, hd] -> [hd, q] via PE identity-transpose, landing in
                # the aoT layout the output projection contracts over
                if qis is None:
                    qis = range(2 * len(pairs))
                for qi in qis:
                    i = c4 * 4 + pairs[0] * 2 + qi
                    pt = psv.tile([128, 128], F16, tag="psv", name="ptr")
                    nc.tensor.matmul(
                        out=pt[:], lhsT=nrm[:, qi, :], rhs=ident_sb[:],
                        is_transpose=True, start=True, stop=True,
                    )
                    if qi % 2 == 0:
                        nc.scalar.copy(
                            out=aoT_sb[:, hp, i * 128:(i + 1) * 128],
                            in_=pt[:])
                    else:
                        nc.vector.tensor_copy(
                            out=aoT_sb[:, hp, i * 128:(i + 1) * 128],
                            in_=pt[:])

            def emit_outproj_st(st, fast_dma=False):
                # one coalesced [128,1024] store per row block (halves the
                # DMA count and semaphore-recycle churn); stores alternate
                # between the Sync HWDGE queue and the idle software-DGE
                # (gpsimd) queue. Completion only gates ysb reuse 4 tiles
                # later. Final stores take the fast Sync path.
                deng = nc.sync if (fast_dma or st % 2 == 0) else nc.gpsimd
                ysb = ypool.tile([128, 2, 512], F16, tag="ysb")
                for nn in range(2):
                    ps = ps512.tile([128, 512], F32, tag="ps512")
                    for hp2 in range(2):
                        nc.tensor.matmul(
                            out=ps[:],
                            lhsT=aoT_sb[:, hp2, st * 128:(st + 1) * 128],
                            rhs=wout_sb[:, hp2,
                                        nn * 512:(nn + 1) * 512],
                            start=(hp2 == 0),
                            stop=(hp2 == 1),
                        )
                    if nn == 0:
                        nc.scalar.copy(out=ysb[:, nn], in_=ps[:])
                    else:
                        nc.vector.tensor_copy(out=ysb[:, nn], in_=ps[:])
                deng.dma_start(
                    out=out_d[st * 128:(st + 1) * 128, :],
                    in_=ysb[:].rearrange("p a b -> p (a b)"),
                )

            # ---- emission schedule: prologue B; per pair-block the PE
            # stream is [scores | filler (prev-block outproj) | AV | B-next
            # chunks] so the PE has independent work while ACT runs exp ----
            # qk chunks 0+1 are emitted kt-major with 8 simultaneously open
            # PSUM groups so the in-order PE consumes each xT k-chunk as it
            # lands instead of stalling a whole m-tile on the last chunk.
            slots = [(ps512, "ps512"), (ps512, "ps512"),
                     (psv, "psv"), (psv, "psv"),
                     (pssc, "pssc"), (pssc, "pssc"),
                     (psav, "psav"), (psav, "psav")]
            qkps = {}
            for ns in range(2):
                for m in range(4):
                    pool, tg = slots[ns * 4 + m]
                    qkps[(ns, m)] = pool.tile([128, 512], F32, tag=tg,
                                              name="qkps")
            for kt in range(8):
                for ns in range(2):
                    for m in range(4):
                        nc.tensor.matmul(
                            out=qkps[(ns, m)][:],
                            lhsT=wqk_sb[:, kt, m * 128:(m + 1) * 128],
                            rhs=xT_sb[:, kt, ns * 512:(ns + 1) * 512],
                            start=(kt == 0),
                            stop=(kt == 7),
                        )
            for ns in range(2):
                for m in range(4):
                    nc.scalar.activation(
                        out=qk_sb[:, m, ns * 512:(ns + 1) * 512],
                        in_=qkps[(ns, m)][:],
                        func=mybir.ActivationFunctionType.Identity,
                        bias=bqk_sb[:, m:m + 1],
                        scale=(SCALE if m < 2 else 1.0),
                    )
            vrr = [(psv, "psv"), (psav, "psav"), (psv, "psv"),
                   (pssc, "pssc"), (psv, "psv")]
            for c, pt in zip((15, 0, 1, 2, 3), vrr):
                emit_v_chunk(c, pool_tag=pt)

            # mid-fillers sized so every block's exp/AV wait is covered by
            # independent PE work; out-projection tiles are deferred toward
            # the late blocks (which have no qk/v B-work left)
            midqk = {(0, 0): [2], (1, 0): [3]}
            midv = {(0, 1): [4, 5, 6, 7], (1, 1): [8, 9, 10, 11],
                    (2, 0): [12, 13], (3, 0): [14, 16]}
            # mid1 fills the scores->exp wait; mid2 fills the recip/nrm
            # wait between AV and the transposes
            mid1 = {(2, 0): [2], (2, 1): [4, 5], (3, 0): [7]}
            mid2 = {(1, 0): [0], (1, 1): [1], (2, 0): [3],
                    (2, 1): [6], (3, 0): [8]}
            for c4 in range(4):
                for hp in range(2):
                    if (c4, hp) == (3, 1):
                        continue
                    ex_big = emit_scores(c4, hp)
                    for qc in midqk.get((c4, hp), []):
                        emit_qk_chunk(qc)
                    for vc in midv.get((c4, hp), []):
                        emit_v_chunk(vc)
                    for st in mid1.get((c4, hp), []):
                        emit_outproj_st(st)
                    nrm = emit_av(c4, hp, ex_big)
                    for st in mid2.get((c4, hp), []):
                        emit_outproj_st(st)
                    emit_transp(c4, hp, nrm)
            # last block at pair granularity; the final transposes go
            # tile-at-a-time so each out-projection row block starts the
            # moment its aoT strip exists
            ex_a = emit_scores(3, 1, pairs=(0,))
            emit_outproj_st(9)
            nrm_a = emit_av(3, 1, ex_a, pairs=(0,))
            emit_outproj_st(10)
            emit_transp(3, 1, nrm_a, pairs=(0,))
            ex_b = emit_scores(3, 1, pairs=(1,))
            emit_outproj_st(11)
            emit_outproj_st(12)
            nrm_b = emit_av(3, 1, ex_b, pairs=(1,))
            emit_outproj_st(13, fast_dma=True)
            emit_transp(3, 1, nrm_b, pairs=(1,), qis=[0])
            emit_outproj_st(14, fast_dma=True)
            emit_transp(3, 1, nrm_b, pairs=(1,), qis=[1])
            emit_outproj_st(15, fast_dma=True)

    nc.compile()
    return nc


_NC = None


def _get_program():
    global _NC
    if _NC is None:
        _NC = _build_program()
    return _NC


def _make_in_maps(x, w_qkv, b_qkv, w_out):
    masks = _build_pair_masks()

    in_maps = []
    for c in range(8):
        b, hg = divmod(c, 4)
        cq = 256 * hg
        wqk = np.concatenate(
            [w_qkv[:, cq:cq + 256], w_qkv[:, 1024 + cq:1024 + cq + 256]],
            axis=1,
        ).astype(np.float16)
        bqk = np.empty((128, 4), np.float32)
        bqk[:, 0] = b_qkv[cq:cq + 128] * SCALE
        bqk[:, 1] = b_qkv[cq + 128:cq + 256] * SCALE
        bqk[:, 2] = b_qkv[1024 + cq:1024 + cq + 128]
        bqk[:, 3] = b_qkv[1024 + cq + 128:1024 + cq + 256]
        in_maps.append({
            "xT": np.ascontiguousarray(x[b].T).astype(np.float16),
            "wqk": wqk,
            "wv": w_qkv[:, 2048 + cq:2048 + cq + 256].astype(np.float16),
            "wout": w_out[cq:cq + 256, :].astype(np.float16),
            "bqk": bqk,
            "masks": masks,
            "ident": np.eye(128, dtype=np.float16),
        })
    return in_maps


def kernel(x, w_qkv, b_qkv, w_out, b_out):
    x = np.asarray(x, np.float32)
    w_qkv = np.asarray(w_qkv, np.float32)
    b_qkv = np.asarray(b_qkv, np.float32)
    w_out = np.asarray(w_out, np.float32)
    b_out = np.asarray(b_out, np.float32)

    in_maps = _make_in_maps(x, w_qkv, b_qkv, w_out)
    nc = _get_program()
    res = run_bass_kernel_spmd(nc, in_maps, list(range(8)))

    b_v = b_qkv[2048:]
    bias_all = b_out + b_v @ w_out  # folds the (untracked) v-bias
    y = np.empty((B, S, D), np.float32)
    for b in range(B):
        acc = np.zeros((S, D), np.float32)
        for hg in range(4):
            acc += res.results[4 * b + hg]["out"].astype(np.float32)
        y[b] = acc + bias_all
    return y
